# revision 3
# baseline (speedup 1.0000x reference)
"""Multi-head causal self-attention on 8 Trainium2 NeuronCores.

Sharding: tensor-parallel over heads. 16 heads / 8 cores = 2 heads per core.
Each core computes Q/K/V projections for its 2 heads (full batch/seq),
causal attention for those heads, and a partial output projection
y_c = O_c @ Wo[:, cols_c].T. The host sums the 8 partials and adds the bias.

Device layout choices (per core):
  - Host feeds x pre-transposed: xT [1024, 4096]  (c, b*t).
  - Q^T, K^T stored [128(d of 2 heads), t] so the S^T = K @ Q^T matmul pair
    packs both heads onto the PE array via row tiling (K=64 each).
  - Scores kept transposed S^T [tk, tq]; softmax without max subtraction
    (|S| <= ~3 for these inputs, exp is safe), denominators via an
    all-ones stationary matmul, normalization after the PV matmul.
  - Causal masking: fully-masked (tk > all tq) tiles skipped; the 4
    diagonal-crossing [128 tk, 512 tq] tiles per query block are masked
    multiplicatively after exp with precomputed 0/1 masks.
"""

import json
import numpy as np

import concourse.bass as bass
import concourse.tile as tile
from concourse import mybir
from concourse.bass_utils import run_bass_kernel_spmd

B, T, C = 2, 2048, 1024
H, D = 16, 64
N_CORES = 8
HPC = H // N_CORES          # heads per core (2)
DPC = HPC * D               # head-dim per core (128)
BT = B * T                  # 4096
KCH = C // 128              # contraction chunks for projections (8)
TQ = 512                    # query-block width (PSUM bank)
TK = 128                    # key-tile height (partitions)
NBLK = T // TQ              # query blocks per batch (4)
F32 = mybir.dt.float32

# ---------------------------------------------------------------------------
# Walrus in this container rejects instructions carrying more than one sync
# wait ("Too many sync wait commands"). Tile's kernel-tail drain carries
# several. Hoist all but the last wait of any instruction onto fresh NoOps
# inserted immediately before it on the same engine (preserves per-engine
# program order, hence semantics).
# ---------------------------------------------------------------------------

def _split_multi_waits(raw: bytes) -> bytes:
    d = json.loads(raw)

    def fix(insts):
        out = []
        for ins in insts:
            waits = (ins.get('sync_info') or {}).get('on_wait') or []
            if len(waits) > 1:
                for i, w in enumerate(waits[:-1]):
                    out.append({
                        'debug': ins.get('debug'),
                        'engine': ins['engine'],
                        'ins': [], 'outs': [],
                        'name': f"{ins['name']}-w{i}",
                        'opcode': 'NoOp',
                        'sync_info': {'on_update': [], 'on_wait': [w]},
                    })
                ins['sync_info']['on_wait'] = waits[-1:]
            out.append(ins)
        return out

    def walk(obj):
        if isinstance(obj, dict):
            if isinstance(obj.get('instructions'), list):
                obj['instructions'] = fix(obj['instructions'])
            for v in obj.values():
                walk(v)
        elif isinstance(obj, list):
            for v in obj:
                walk(v)

    for f in d.get('functions', []):
        walk(f.get('blocks'))
    return json.dumps(d).encode()


def _install_bir_patch(nc):
    orig = nc.to_json_bytes
    nc.to_json_bytes = lambda: _split_multi_waits(orig())


# ---------------------------------------------------------------------------
# Device kernel (SPMD; per-core inputs differ only in weight slices)
# ---------------------------------------------------------------------------

def build_kernel(nreps=1):
    nc = bass.Bass("TRN2", target_bir_lowering=False, debug=False)
    xt = nc.dram_tensor("xt", [C, BT], F32, kind="ExternalInput").ap()
    wq = nc.dram_tensor("wq", [C, DPC], F32, kind="ExternalInput").ap()
    wk = nc.dram_tensor("wk", [C, DPC], F32, kind="ExternalInput").ap()
    wv = nc.dram_tensor("wv", [C, DPC], F32, kind="ExternalInput").ap()
    wo = nc.dram_tensor("wo", [DPC, C], F32, kind="ExternalInput").ap()
    msk = nc.dram_tensor("mask", [4, TK, TQ], F32, kind="ExternalInput").ap()
    y = nc.dram_tensor("y", [BT, C], F32, kind="ExternalOutput").ap()

    xt_r = xt.rearrange("(k p) t -> p k t", p=128)          # [128, 8, 4096]
    wq_r = wq.rearrange("(k p) d -> p k d", p=128)          # [128, 8, 128]
    wk_r = wk.rearrange("(k p) d -> p k d", p=128)
    wv_r = wv.rearrange("(k p) d -> p k d", p=128)
    y_r = y.rearrange("(blk m p) c -> blk p m c", m=4, p=128)  # [8, 128, 4, 1024]

    with tile.TileContext(nc) as tc:
        for _ in range(nreps):
            _build_body(nc, tc, xt_r, wq_r, wk_r, wv_r, wo, msk, y_r)
    _install_bir_patch(nc)
    return nc


def _build_body(nc, tc, xt_r, wq_r, wk_r, wv_r, wo, msk, y_r):
    from contextlib import ExitStack
    ctx = ExitStack()
    with ctx:
        const = ctx.enter_context(tc.tile_pool(name="const", bufs=1))
        xt_pool = ctx.enter_context(tc.tile_pool(name="xt", bufs=2))
        qkv = ctx.enter_context(tc.tile_pool(name="qkv", bufs=1))
        p_pool = ctx.enter_context(tc.tile_pool(name="p", bufs=4))
        epi = ctx.enter_context(tc.tile_pool(name="epi", bufs=2))
        ystage = ctx.enter_context(tc.tile_pool(name="ystage", bufs=2))
        ps_misc = ctx.enter_context(tc.tile_pool(name="ps_misc", bufs=2, space="PSUM"))
        ps_s = ctx.enter_context(tc.tile_pool(name="ps_s", bufs=4, space="PSUM"))
        ps_o = ctx.enter_context(tc.tile_pool(name="ps_o", bufs=1, space="PSUM"))
        ps_d = ctx.enter_context(tc.tile_pool(name="ps_d", bufs=1, space="PSUM"))

        # --- constants ---
        wq_sb = const.tile([128, KCH, DPC], F32, tag="wq")
        wk_sb = const.tile([128, KCH, DPC], F32, tag="wk")
        wv_sb = const.tile([128, KCH, DPC], F32, tag="wv")
        wo_sb = const.tile([128, C], F32, tag="wo")
        mask_sb = const.tile([128, 4, TQ], F32, tag="mask")
        ones_sb = const.tile([128, 64], F32, tag="ones")
        nc.sync.dma_start(wq_sb[:], wq_r[:])
        nc.sync.dma_start(wk_sb[:], wk_r[:])
        nc.sync.dma_start(wv_sb[:], wv_r[:])
        nc.sync.dma_start(wo_sb[:], wo[:])
        nc.sync.dma_start(mask_sb[:], msk.rearrange("j p q -> p j q"))
        nc.vector.memset(ones_sb[:], 1.0)

        # --- persistent Q^T / K^T / V tiles (both batches) ---
        qt_sb = qkv.tile([128, B, T], F32, tag="qt")   # [d(2 heads), b, t]
        kt_sb = qkv.tile([128, B, T], F32, tag="kt")
        v_sb = qkv.tile([128, B, T], F32, tag="v")     # [t%128, b, 128*(t//128)+d]

        # --- phase 1: projections ---
        for b in range(B):
            for tchunk in range(T // TQ):
                t0 = b * T + tchunk * TQ
                xt_sb = xt_pool.tile([128, KCH, TQ], F32, tag="xt")
                nc.sync.dma_start(xt_sb[:], xt_r[:, :, t0:t0 + TQ])

                for w_sb, dst in ((wq_sb, qt_sb), (wk_sb, kt_sb)):
                    ps = ps_misc.tile([128, TQ], F32, tag="misc")
                    for k in range(KCH):
                        nc.tensor.matmul(ps[:], w_sb[:, k, :], xt_sb[:, k, :],
                                         start=(k == 0), stop=(k == KCH - 1))
                    nc.vector.tensor_copy(
                        dst[:, b, tchunk * TQ:(tchunk + 1) * TQ], ps[:])

                ps = ps_misc.tile([128, TQ], F32, tag="misc")
                for m in range(4):
                    for k in range(KCH):
                        nc.tensor.matmul(ps[:, m * 128:(m + 1) * 128],
                                         xt_sb[:, k, m * 128:(m + 1) * 128],
                                         wv_sb[:, k, :],
                                         start=(k == 0), stop=(k == KCH - 1))
                nc.vector.tensor_copy(
                    v_sb[:, b, tchunk * TQ:(tchunk + 1) * TQ], ps[:])

        # --- phase 2: attention + partial out-projection ---
        for b in range(B):
            for i in range(NBLK):
                q0 = i * TQ
                njt = 4 * i + 4           # needed key tiles (causal)
                o_ps = ps_o.tile([128, TQ], F32, tag="o")
                d_ps = ps_d.tile([128, TQ], F32, tag="d")
                for j in range(njt):
                    s_a = ps_s.tile([128, TQ], F32, tag="s")
                    s_b = ps_s.tile([128, TQ], F32, tag="s")
                    k0 = j * TK
                    nc.tensor.matmul(s_a[:], kt_sb[0:64, b, k0:k0 + TK],
                                     qt_sb[0:64, b, q0:q0 + TQ])
                    nc.tensor.matmul(s_b[:], kt_sb[64:128, b, k0:k0 + TK],
                                     qt_sb[64:128, b, q0:q0 + TQ])
                    p_a = p_pool.tile([128, TQ], F32, tag="p")
                    p_b = p_pool.tile([128, TQ], F32, tag="p")
                    nc.scalar.activation(p_a[:], s_a[:],
                                         mybir.ActivationFunctionType.Exp,
                                         scale=0.125)
                    nc.scalar.activation(p_b[:], s_b[:],
                                         mybir.ActivationFunctionType.Exp,
                                         scale=0.125)
                    if j >= 4 * i:        # diagonal-crossing tile
                        jj = j - 4 * i
                        nc.vector.tensor_mul(p_a[:], p_a[:], mask_sb[:, jj, :])
                        nc.vector.tensor_mul(p_b[:], p_b[:], mask_sb[:, jj, :])
                    fl = (j == 0)
                    ll = (j == njt - 1)
                    nc.tensor.matmul(o_ps[0:64, :], v_sb[:, b, k0:k0 + 64],
                                     p_a[:], start=fl, stop=ll)
                    nc.tensor.matmul(o_ps[64:128, :], v_sb[:, b, k0 + 64:k0 + TK],
                                     p_b[:], start=fl, stop=ll)
                    nc.tensor.matmul(d_ps[0:64, :], ones_sb[:, 0:64],
                                     p_a[:], start=fl, stop=ll)
                    nc.tensor.matmul(d_ps[64:128, :], ones_sb[:, 0:64],
                                     p_b[:], start=fl, stop=ll)

                rec = epi.tile([128, TQ], F32, tag="rec")
                nc.vector.reciprocal(rec[:], d_ps[:])
                o_n = epi.tile([128, TQ], F32, tag="on")
                nc.vector.tensor_mul(o_n[:], o_ps[:], rec[:])

                y_sb = ystage.tile([128, 4, C], F32, tag="y")
                for m in range(4):
                    for n in range(2):
                        y_ps = ps_misc.tile([128, TQ], F32, tag="misc")
                        nc.tensor.matmul(y_ps[:], o_n[:, m * 128:(m + 1) * 128],
                                         wo_sb[:, n * TQ:(n + 1) * TQ])
                        nc.vector.tensor_copy(
                            y_sb[:, m, n * TQ:(n + 1) * TQ], y_ps[:])
                nc.sync.dma_start(y_r[b * NBLK + i], y_sb[:])


# ---------------------------------------------------------------------------
# Host wrapper
# ---------------------------------------------------------------------------

_CACHE = {}


def _prep_inputs(x, Wq, Wk, Wv, Wo):
    xt = np.ascontiguousarray(x.reshape(BT, C).T)            # [C, BT]
    mask = np.zeros((4, TK, TQ), np.float32)
    for jj in range(4):
        for p in range(TK):
            lo = 128 * jj + p
            if lo < TQ:
                mask[jj, p, lo:] = 1.0
    in_maps = []
    for c in range(N_CORES):
        r0 = c * DPC
        in_maps.append({
            "xt": xt,
            "wq": np.ascontiguousarray(Wq[r0:r0 + DPC, :].T),
            "wk": np.ascontiguousarray(Wk[r0:r0 + DPC, :].T),
            "wv": np.ascontiguousarray(Wv[r0:r0 + DPC, :].T),
            "wo": np.ascontiguousarray(Wo[:, r0:r0 + DPC].T),
            "mask": mask,
        })
    return in_maps


def kernel(x, Wq, Wk, Wv, Wo, bo):
    x = np.asarray(x, np.float32)
    Wq = np.asarray(Wq, np.float32)
    Wk = np.asarray(Wk, np.float32)
    Wv = np.asarray(Wv, np.float32)
    Wo = np.asarray(Wo, np.float32)
    bo = np.asarray(bo, np.float32)

    if "nc" not in _CACHE:
        _CACHE["nc"] = build_kernel()
    nc = _CACHE["nc"]

    in_maps = _prep_inputs(x, Wq, Wk, Wv, Wo)
    res = run_bass_kernel_spmd(nc, in_maps, core_ids=list(range(N_CORES)))
    acc = np.zeros((BT, C), np.float64)
    for r in res.results:
        acc += r["y"]
    out = (acc + bo).astype(np.float32)
    return out.reshape(B, T, C)


# revision 10
# speedup vs baseline: 1.4701x; 1.4701x over previous
"""Multi-head causal self-attention on 8 Trainium2 NeuronCores.

Sharding: tensor-parallel over heads. 16 heads / 8 cores = 2 heads per core.
Each core computes Q/K/V projections for its 2 heads (full batch/seq),
causal attention for those heads, and a partial output projection
y_c = O_c @ Wo[:, cols_c].T. The host sums the 8 partials and adds the bias.

Device layout choices (per core):
  - Host feeds x pre-transposed: xT [1024, 4096]  (c, b*t).
  - Q^T, K^T stored [128(d of 2 heads), t] so the S^T = K @ Q^T matmul pair
    packs both heads onto the PE array via row tiling (K=64 each).
  - Scores kept transposed S^T [tk, tq]; softmax without max subtraction
    (|S| <= ~3 for these inputs, exp is safe), denominators via an
    all-ones stationary matmul, normalization after the PV matmul.
  - Causal masking: fully-masked (tk > all tq) tiles skipped; the 4
    diagonal-crossing [128 tk, 512 tq] tiles per query block are masked
    multiplicatively after exp with precomputed 0/1 masks.
"""

import json
import numpy as np

import concourse.bass as bass
import concourse.tile as tile
from concourse import mybir
from concourse.bass_utils import run_bass_kernel_spmd

B, T, C = 2, 2048, 1024
H, D = 16, 64
N_CORES = 8
HPC = H // N_CORES          # heads per core (2)
DPC = HPC * D               # head-dim per core (128)
BT = B * T                  # 4096
KCH = C // 128              # contraction chunks for projections (8)
TQ = 512                    # query-block width (PSUM bank)
TK = 128                    # key-tile height (partitions)
NBLK = T // TQ              # query blocks per batch (4)
F32 = mybir.dt.float32

# ---------------------------------------------------------------------------
# Walrus in this container rejects instructions carrying more than one sync
# wait ("Too many sync wait commands"). Tile's kernel-tail drain carries
# several. Hoist all but the last wait of any instruction onto fresh NoOps
# inserted immediately before it on the same engine (preserves per-engine
# program order, hence semantics).
# ---------------------------------------------------------------------------

def _split_multi_waits(raw: bytes) -> bytes:
    d = json.loads(raw)

    def fix(insts):
        out = []
        for ins in insts:
            waits = (ins.get('sync_info') or {}).get('on_wait') or []
            if len(waits) > 1:
                for i, w in enumerate(waits[:-1]):
                    out.append({
                        'debug': ins.get('debug'),
                        'engine': ins['engine'],
                        'ins': [], 'outs': [],
                        'name': f"{ins['name']}-w{i}",
                        'opcode': 'NoOp',
                        'sync_info': {'on_update': [], 'on_wait': [w]},
                    })
                ins['sync_info']['on_wait'] = waits[-1:]
            out.append(ins)
        return out

    def walk(obj):
        if isinstance(obj, dict):
            if isinstance(obj.get('instructions'), list):
                obj['instructions'] = fix(obj['instructions'])
            for v in obj.values():
                walk(v)
        elif isinstance(obj, list):
            for v in obj:
                walk(v)

    for f in d.get('functions', []):
        walk(f.get('blocks'))
    return json.dumps(d).encode()


def _install_bir_patch(nc):
    orig = nc.to_json_bytes
    nc.to_json_bytes = lambda: _split_multi_waits(orig())


# ---------------------------------------------------------------------------
# Device kernel (SPMD; per-core inputs differ only in weight slices)
# ---------------------------------------------------------------------------

def build_kernel(nreps=1):
    nc = bass.Bass("TRN2", target_bir_lowering=False, debug=False)
    xt = nc.dram_tensor("xt", [C, BT], F32, kind="ExternalInput").ap()
    wq = nc.dram_tensor("wq", [C, DPC], F32, kind="ExternalInput").ap()
    wk = nc.dram_tensor("wk", [C, DPC], F32, kind="ExternalInput").ap()
    wv = nc.dram_tensor("wv", [C, DPC], F32, kind="ExternalInput").ap()
    wo = nc.dram_tensor("wo", [DPC, C], F32, kind="ExternalInput").ap()
    msk = nc.dram_tensor("mask", [4, TK, TQ], F32, kind="ExternalInput").ap()
    one = nc.dram_tensor("ones", [128, 128], F32, kind="ExternalInput").ap()
    y = nc.dram_tensor("y", [BT, C], F32, kind="ExternalOutput").ap()

    xt_r = xt.rearrange("(k p) t -> p k t", p=128)          # [128, 8, 4096]
    wq_r = wq.rearrange("(k p) d -> p k d", p=128)          # [128, 8, 128]
    wk_r = wk.rearrange("(k p) d -> p k d", p=128)
    wv_r = wv.rearrange("(k p) d -> p k d", p=128)
    y_r = y.rearrange("(blk m p) c -> blk p m c", m=4, p=128)  # [8, 128, 4, 1024]

    with tile.TileContext(nc) as tc:
        for _ in range(nreps):
            _build_body(nc, tc, xt_r, wq_r, wk_r, wv_r, wo, msk, one, y_r)
    _install_bir_patch(nc)
    return nc


def _build_body(nc, tc, xt_r, wq_r, wk_r, wv_r, wo, msk, one, y_r):
    from contextlib import ExitStack
    from concourse.masks import make_identity

    F32R = mybir.dt.float32r

    def r(ap):
        return ap.bitcast(F32R)

    ctx = ExitStack()
    with ctx:
        const = ctx.enter_context(tc.tile_pool(name="const", bufs=1))
        xt_pool = ctx.enter_context(tc.tile_pool(name="xt", bufs=2))
        qkv = ctx.enter_context(tc.tile_pool(name="qkv", bufs=1))
        vt_pool = ctx.enter_context(tc.tile_pool(name="vt", bufs=2))
        p_pool = ctx.enter_context(tc.tile_pool(name="p", bufs=3))
        epi = ctx.enter_context(tc.tile_pool(name="epi", bufs=2))
        ystage = ctx.enter_context(tc.tile_pool(name="ystage", bufs=2))
        # 8 PSUM banks total: s-pool 2 slots x 2 banks (also serves the
        # 1-bank proj/out-proj tiles), o and d 1 slot x 2 banks each.
        ps_s = ctx.enter_context(tc.tile_pool(name="ps_s", bufs=2, space="PSUM"))
        ps_o = ctx.enter_context(tc.tile_pool(name="ps_o", bufs=1, space="PSUM"))
        ps_d = ctx.enter_context(tc.tile_pool(name="ps_d", bufs=1, space="PSUM"))

        # --- constants ---
        wq_sb = const.tile([128, KCH, DPC], F32R, tag="wq")
        wk_sb = const.tile([128, KCH, DPC], F32R, tag="wk")
        wv_sb = const.tile([128, KCH, DPC], F32R, tag="wv")
        wo_sb = const.tile([128, C], F32R, tag="wo")
        mask_sb = const.tile([128, 4, TQ], F32R, tag="mask")
        ones_sb = const.tile([128, 128], F32R, tag="ones")
        ident_sb = const.tile([128, 128], F32, tag="ident")
        nc.sync.dma_start(wq_sb[:], r(wq_r[:]))
        nc.sync.dma_start(wk_sb[:], r(wk_r[:]))
        nc.sync.dma_start(wv_sb[:], r(wv_r[:]))
        nc.sync.dma_start(wo_sb[:], r(wo[:]))
        nc.sync.dma_start(mask_sb[:], r(msk.rearrange("j p q -> p j q")))
        nc.sync.dma_start(ones_sb[:], r(one[:]))
        make_identity(nc, ident_sb[:])

        # --- persistent Q^T / K^T / V tiles (both batches) ---
        qt_sb = qkv.tile([128, B, T], F32R, tag="qt")   # [d(2 heads), b, t]
        kt_sb = qkv.tile([128, B, T], F32R, tag="kt")
        v_sb = qkv.tile([128, B, T], F32R, tag="v")     # [t%128, b, 128*(t//128)+d]

        # --- phase 1: projections ---
        for b in range(B):
            for tchunk in range(T // TQ):
                t0 = b * T + tchunk * TQ
                xt_sb = xt_pool.tile([128, KCH, TQ], F32R, tag="xt")
                nc.sync.dma_start(xt_sb[:], r(xt_r[:, :, t0:t0 + TQ]))

                for w_sb, dst in ((wq_sb, qt_sb), (wk_sb, kt_sb)):
                    ps = ps_s.tile([128, TQ], F32, tag="s")
                    for k in range(KCH):
                        nc.tensor.matmul(ps[:], r(w_sb[:, k, :]), xt_sb[:, k, :],
                                         start=(k == 0), stop=(k == KCH - 1))
                    nc.vector.tensor_copy(
                        dst[:, b, tchunk * TQ:(tchunk + 1) * TQ], ps[:])

                # V^T in PSUM, copy to SBUF, then PE-transpose to [t, d]
                ps = ps_s.tile([128, TQ], F32, tag="s")
                for k in range(KCH):
                    nc.tensor.matmul(ps[:], wv_sb[:, k, :], xt_sb[:, k, :],
                                     start=(k == 0), stop=(k == KCH - 1))
                vt_sb = vt_pool.tile([128, TQ], F32, tag="vt")
                nc.vector.tensor_copy(vt_sb[:], ps[:])
                ps = ps_s.tile([128, TQ], F32, tag="s")
                for m in range(4):
                    nc.tensor.transpose(ps[:, m * 128:(m + 1) * 128],
                                        vt_sb[:, m * 128:(m + 1) * 128],
                                        ident_sb[:])
                nc.vector.tensor_copy(
                    v_sb[:, b, tchunk * TQ:(tchunk + 1) * TQ], ps[:])

        # --- phase 2: attention + partial out-projection ---
        for b in range(B):
            for i in range(NBLK):
                q0 = i * TQ
                njt = 4 * i + 4           # needed key tiles (causal)
                o_ps = ps_o.tile([128, 2, TQ], F32, tag="o")
                d_ps = ps_d.tile([128, 2, TQ], F32, tag="d")
                for j in range(njt):
                    # S^T pair: head A on PE rows 0-63 -> psum half 0, head B
                    # on rows 64-127 -> half 1 (row-tiled, runs concurrently).
                    # One exp covers both heads (1024-wide batch).
                    k0 = j * TK
                    s_ps = ps_s.tile([128, 2, TQ], F32, tag="s")
                    p_sb = p_pool.tile([128, 2, TQ], F32R, tag="p")
                    nc.tensor.matmul(s_ps[:, 0, :],
                                     kt_sb[0:64, b, k0:k0 + TK],
                                     qt_sb[0:64, b, q0:q0 + TQ])
                    nc.tensor.matmul(s_ps[:, 1, :],
                                     kt_sb[64:128, b, k0:k0 + TK],
                                     qt_sb[64:128, b, q0:q0 + TQ])
                    nc.scalar.activation(p_sb[:], s_ps[:],
                                         mybir.ActivationFunctionType.Exp,
                                         scale=0.125)
                    if j >= 4 * i:            # diagonal-crossing tile
                        jj = j - 4 * i
                        nc.vector.tensor_mul(p_sb[:, 0, :], p_sb[:, 0, :],
                                             mask_sb[:, jj, :])
                        nc.vector.tensor_mul(p_sb[:, 1, :], p_sb[:, 1, :],
                                             mask_sb[:, jj, :])
                    fl = (j == 0)
                    ll = (j == njt - 1)
                    # float32r requires full 128-column stationaries, so both
                    # heads use the same [V_A|V_B] weights; each head's valid
                    # rows are its own d-range, the other half is ignored.
                    nc.tensor.matmul(o_ps[:, 0, :],
                                     v_sb[:, b, k0:k0 + TK],
                                     p_sb[:, 0, :], start=fl, stop=ll)
                    nc.tensor.matmul(o_ps[:, 1, :],
                                     v_sb[:, b, k0:k0 + TK],
                                     p_sb[:, 1, :], start=fl, stop=ll)
                    nc.tensor.matmul(d_ps[:, 0, :], ones_sb[:],
                                     p_sb[:, 0, :], start=fl, stop=ll)
                    nc.tensor.matmul(d_ps[:, 1, :], ones_sb[:],
                                     p_sb[:, 1, :], start=fl, stop=ll)

                rec = epi.tile([128, TQ], F32, tag="rec")
                nc.vector.reciprocal(rec[0:64, :], d_ps[0:64, 0, :])
                nc.vector.reciprocal(rec[64:128, :], d_ps[64:128, 1, :])
                o_n = epi.tile([128, TQ], F32R, tag="on")
                nc.vector.tensor_mul(o_n[0:64, :], o_ps[0:64, 0, :],
                                     rec[0:64, :])
                nc.vector.tensor_mul(o_n[64:128, :], o_ps[64:128, 1, :],
                                     rec[64:128, :])

                y_sb = ystage.tile([128, 4, C], F32, tag="y")
                for m in range(4):
                    for n in range(2):
                        y_ps = ps_s.tile([128, TQ], F32, tag="s")
                        nc.tensor.matmul(y_ps[:], r(o_n[:, m * 128:(m + 1) * 128]),
                                         r(wo_sb[:, n * TQ:(n + 1) * TQ]))
                        if (m + n) % 2 == 0:
                            nc.vector.tensor_copy(
                                y_sb[:, m, n * TQ:(n + 1) * TQ], y_ps[:])
                        else:
                            nc.scalar.copy(
                                y_sb[:, m, n * TQ:(n + 1) * TQ], y_ps[:])
                nc.sync.dma_start(y_r[b * NBLK + i], y_sb[:])


# ---------------------------------------------------------------------------
# Host wrapper
# ---------------------------------------------------------------------------

_CACHE = {}


def _prep_inputs(x, Wq, Wk, Wv, Wo):
    xt = np.ascontiguousarray(x.reshape(BT, C).T)            # [C, BT]
    mask = np.zeros((4, TK, TQ), np.float32)
    for jj in range(4):
        for p in range(TK):
            lo = 128 * jj + p
            if lo < TQ:
                mask[jj, p, lo:] = 1.0
    in_maps = []
    for c in range(N_CORES):
        r0 = c * DPC
        in_maps.append({
            "xt": xt,
            "wq": np.ascontiguousarray(Wq[r0:r0 + DPC, :].T),
            "wk": np.ascontiguousarray(Wk[r0:r0 + DPC, :].T),
            "wv": np.ascontiguousarray(Wv[r0:r0 + DPC, :].T),
            "wo": np.ascontiguousarray(Wo[:, r0:r0 + DPC].T),
            "mask": mask,
            "ones": np.ones((128, 128), np.float32),
        })
    return in_maps


def kernel(x, Wq, Wk, Wv, Wo, bo):
    x = np.asarray(x, np.float32)
    Wq = np.asarray(Wq, np.float32)
    Wk = np.asarray(Wk, np.float32)
    Wv = np.asarray(Wv, np.float32)
    Wo = np.asarray(Wo, np.float32)
    bo = np.asarray(bo, np.float32)

    if "nc" not in _CACHE:
        _CACHE["nc"] = build_kernel()
    nc = _CACHE["nc"]

    in_maps = _prep_inputs(x, Wq, Wk, Wv, Wo)
    res = run_bass_kernel_spmd(nc, in_maps, core_ids=list(range(N_CORES)))
    acc = np.zeros((BT, C), np.float64)
    for r in res.results:
        acc += r["y"]
    out = (acc + bo).astype(np.float32)
    return out.reshape(B, T, C)


# revision 15
# speedup vs baseline: 2.2380x; 1.5224x over previous
"""Multi-head causal self-attention on 8 Trainium2 NeuronCores.

Sharding: tensor-parallel over heads. 16 heads / 8 cores = 2 heads per core.
Each core computes Q/K/V projections for its 2 heads (full batch/seq),
causal attention for those heads, and a partial output projection
y_c = O_c @ Wo[:, cols_c].T. The host sums the 8 partials and adds the bias.

Device layout choices (per core):
  - Host feeds x pre-transposed: xT [1024, 4096]  (c, b*t).
  - Q^T, K^T stored [128(d of 2 heads), t] so the S^T = K @ Q^T matmul pair
    packs both heads onto the PE array via row tiling (K=64 each).
  - Scores kept transposed S^T [tk, tq]; softmax without max subtraction
    (|S| <= ~3 for these inputs, exp is safe), denominators via an
    all-ones stationary matmul, normalization after the PV matmul.
  - Causal masking: fully-masked (tk > all tq) tiles skipped; the 4
    diagonal-crossing [128 tk, 512 tq] tiles per query block are masked
    multiplicatively after exp with precomputed 0/1 masks.
"""

import json
import numpy as np

import concourse.bass as bass
import concourse.tile as tile
from concourse import mybir
from concourse.bass_utils import run_bass_kernel_spmd

B, T, C = 2, 2048, 1024
H, D = 16, 64
N_CORES = 8
HPC = H // N_CORES          # heads per core (2)
DPC = HPC * D               # head-dim per core (128)
BT = B * T                  # 4096
KCH = C // 128              # contraction chunks for projections (8)
TQ = 512                    # query-block width (PSUM bank)
TK = 128                    # key-tile height (partitions)
NBLK = T // TQ              # query blocks per batch (4)
F32 = mybir.dt.float32
BF16 = mybir.dt.bfloat16

# ---------------------------------------------------------------------------
# Walrus in this container rejects instructions carrying more than one sync
# wait ("Too many sync wait commands"). Tile's kernel-tail drain carries
# several. Hoist all but the last wait of any instruction onto fresh NoOps
# inserted immediately before it on the same engine (preserves per-engine
# program order, hence semantics).
# ---------------------------------------------------------------------------

def _split_multi_waits(raw: bytes) -> bytes:
    d = json.loads(raw)

    def fix(insts):
        out = []
        for ins in insts:
            waits = (ins.get('sync_info') or {}).get('on_wait') or []
            if len(waits) > 1:
                for i, w in enumerate(waits[:-1]):
                    out.append({
                        'debug': ins.get('debug'),
                        'engine': ins['engine'],
                        'ins': [], 'outs': [],
                        'name': f"{ins['name']}-w{i}",
                        'opcode': 'NoOp',
                        'sync_info': {'on_update': [], 'on_wait': [w]},
                    })
                ins['sync_info']['on_wait'] = waits[-1:]
            out.append(ins)
        return out

    def walk(obj):
        if isinstance(obj, dict):
            if isinstance(obj.get('instructions'), list):
                obj['instructions'] = fix(obj['instructions'])
            for v in obj.values():
                walk(v)
        elif isinstance(obj, list):
            for v in obj:
                walk(v)

    for f in d.get('functions', []):
        walk(f.get('blocks'))
    return json.dumps(d).encode()


def _install_bir_patch(nc):
    orig = nc.to_json_bytes
    nc.to_json_bytes = lambda: _split_multi_waits(orig())


# ---------------------------------------------------------------------------
# Device kernel (SPMD; per-core inputs differ only in weight slices)
# ---------------------------------------------------------------------------

def build_kernel(nreps=1, phases=('proj', 'attn', 'out')):
    nc = bass.Bass("TRN2", target_bir_lowering=False, debug=False)
    xt = nc.dram_tensor("xt", [C, BT], F32, kind="ExternalInput").ap()
    wq = nc.dram_tensor("wq", [C, DPC], F32, kind="ExternalInput").ap()
    wk = nc.dram_tensor("wk", [C, DPC], F32, kind="ExternalInput").ap()
    wv = nc.dram_tensor("wv", [C, DPC], F32, kind="ExternalInput").ap()
    wo = nc.dram_tensor("wo", [DPC, C], F32, kind="ExternalInput").ap()
    msk = nc.dram_tensor("mask", [4, TK, TQ], BF16, kind="ExternalInput").ap()
    one = nc.dram_tensor("ones", [128, 64], BF16, kind="ExternalInput").ap()
    y = nc.dram_tensor("y", [BT, C], F32, kind="ExternalOutput").ap()

    xt_r = xt.rearrange("(k p) t -> p k t", p=128)          # [128, 8, 4096]
    wq_r = wq.rearrange("(k p) d -> p k d", p=128)          # [128, 8, 128]
    wk_r = wk.rearrange("(k p) d -> p k d", p=128)
    wv_r = wv.rearrange("(k p) d -> p k d", p=128)
    y_r = y.rearrange("(blk m p) c -> blk p m c", m=4, p=128)  # [8, 128, 4, 1024]

    with tile.TileContext(nc) as tc:
        for _ in range(nreps):
            _build_body(nc, tc, xt_r, wq_r, wk_r, wv_r, wo, msk, one, y_r, phases)
    _install_bir_patch(nc)
    return nc


def _build_body(nc, tc, xt_r, wq_r, wk_r, wv_r, wo, msk, one, y_r, phases=('proj', 'attn', 'out')):
    from contextlib import ExitStack
    from concourse.masks import make_identity

    F32R = mybir.dt.float32r

    def r(ap):
        return ap.bitcast(F32R)

    ctx = ExitStack()
    with ctx:
        const = ctx.enter_context(tc.tile_pool(name="const", bufs=1))
        xt_pool = ctx.enter_context(tc.tile_pool(name="xt", bufs=2))
        qkv = ctx.enter_context(tc.tile_pool(name="qkv", bufs=1))
        vt_pool = ctx.enter_context(tc.tile_pool(name="vt", bufs=2))
        p_pool = ctx.enter_context(tc.tile_pool(name="p", bufs=3))
        epi = ctx.enter_context(tc.tile_pool(name="epi", bufs=2))
        ystage = ctx.enter_context(tc.tile_pool(name="ystage", bufs=2))
        # 8 PSUM banks total: s-pool 2 slots x 2 banks (also serves the
        # 1-bank proj/out-proj tiles), o and d 1 slot x 2 banks each.
        ps_s = ctx.enter_context(tc.tile_pool(name="ps_s", bufs=2, space="PSUM"))
        ps_o = ctx.enter_context(tc.tile_pool(name="ps_o", bufs=1, space="PSUM"))
        ps_d = ctx.enter_context(tc.tile_pool(name="ps_d", bufs=1, space="PSUM"))
        ps_y = ctx.enter_context(tc.tile_pool(name="ps_y", bufs=2, space="PSUM"))

        # --- constants ---
        wq_sb = const.tile([128, KCH, DPC], F32R, tag="wq")
        wk_sb = const.tile([128, KCH, DPC], F32R, tag="wk")
        wv_sb = const.tile([128, KCH, DPC], F32R, tag="wv")
        wo_sb = const.tile([128, C], F32R, tag="wo")
        mask_sb = const.tile([128, 4, TQ], BF16, tag="mask")
        ones_sb = const.tile([128, 64], BF16, tag="ones")
        ident_sb = const.tile([128, 128], F32, tag="ident")
        nc.sync.dma_start(wq_sb[:], r(wq_r[:]))
        nc.sync.dma_start(wk_sb[:], r(wk_r[:]))
        nc.sync.dma_start(wv_sb[:], r(wv_r[:]))
        nc.sync.dma_start(wo_sb[:], r(wo[:]))
        nc.sync.dma_start(mask_sb[:], msk.rearrange("j p q -> p j q"))
        nc.sync.dma_start(ones_sb[:], one[:])
        make_identity(nc, ident_sb[:])

        # --- persistent Q^T / K^T / V tiles, separate per batch so batch
        # b+1's projections overlap batch b's attention ---
        qt_b = [qkv.tile([128, T], F32R, name=f"qt{b}", tag=f"qt{b}") for b in range(B)]
        kt_b = [qkv.tile([128, T], F32R, name=f"kt{b}", tag=f"kt{b}") for b in range(B)]
        v_b = [qkv.tile([128, T], BF16, name=f"v{b}", tag=f"v{b}") for b in range(B)]

        for b in range(B):
            qt_sb, kt_sb, v_sb = qt_b[b], kt_b[b], v_b[b]
            # --- projections for batch b ---
            for tchunk in range(T // TQ) if 'proj' in phases else []:
                t0 = b * T + tchunk * TQ
                xt_sb = xt_pool.tile([128, KCH, TQ], F32R, tag="xt")
                nc.sync.dma_start(xt_sb[:], r(xt_r[:, :, t0:t0 + TQ]))

                for w_sb, dst in ((wq_sb, qt_sb), (wk_sb, kt_sb)):
                    ps = ps_s.tile([128, TQ], F32, tag="s")
                    for k in range(KCH):
                        nc.tensor.matmul(ps[:], r(w_sb[:, k, :]), xt_sb[:, k, :],
                                         start=(k == 0), stop=(k == KCH - 1))
                    nc.vector.tensor_copy(
                        dst[:, tchunk * TQ:(tchunk + 1) * TQ], ps[:])

                # V^T in PSUM, copy to SBUF, then PE-transpose to [t, d]
                ps = ps_s.tile([128, TQ], F32, tag="s")
                for k in range(KCH):
                    nc.tensor.matmul(ps[:], wv_sb[:, k, :], xt_sb[:, k, :],
                                     start=(k == 0), stop=(k == KCH - 1))
                vt_sb = vt_pool.tile([128, TQ], F32, tag="vt")
                nc.vector.tensor_copy(vt_sb[:], ps[:])
                ps = ps_s.tile([128, TQ], F32, tag="s")
                for m in range(4):
                    nc.tensor.transpose(ps[:, m * 128:(m + 1) * 128],
                                        vt_sb[:, m * 128:(m + 1) * 128],
                                        ident_sb[:])
                nc.vector.tensor_copy(
                    v_sb[:, tchunk * TQ:(tchunk + 1) * TQ], ps[:])

            # --- attention + partial out-projection for batch b ---
            for i in range(NBLK) if 'attn' in phases else []:
                q0 = i * TQ
                njt = 4 * i + 4           # needed key tiles (causal)
                o_ps = ps_o.tile([128, TQ], F32, tag="o")
                d_ps = ps_d.tile([128, TQ], F32, tag="d")
                for j in range(njt):
                    # S^T pair: head A on PE rows 0-63 -> psum half 0, head B
                    # on rows 64-127 -> half 1 (row-tiled, runs concurrently).
                    # One exp covers both heads (1024-wide batch).
                    k0 = j * TK
                    s_ps = ps_s.tile([128, 2, TQ], F32, tag="s")
                    p_sb = p_pool.tile([128, 2, TQ], BF16, tag="p")
                    nc.tensor.matmul(s_ps[:, 0, :],
                                     kt_sb[0:64, k0:k0 + TK],
                                     qt_sb[0:64, q0:q0 + TQ])
                    nc.tensor.matmul(s_ps[:, 1, :],
                                     kt_sb[64:128, k0:k0 + TK],
                                     qt_sb[64:128, q0:q0 + TQ])
                    nc.scalar.activation(p_sb[:], s_ps[:],
                                         mybir.ActivationFunctionType.Exp,
                                         scale=0.125)
                    if j >= 4 * i:            # diagonal-crossing tile
                        jj = j - 4 * i
                        nc.vector.tensor_mul(p_sb[:, 0, :], p_sb[:, 0, :],
                                             mask_sb[:, jj, :])
                        nc.vector.tensor_mul(p_sb[:, 1, :], p_sb[:, 1, :],
                                             mask_sb[:, jj, :])
                    fl = (j == 0)
                    ll = (j == njt - 1)
                    # bf16 PV + denominator matmuls, column-tiled so the two
                    # heads run concurrently on separate PE column halves.
                    nc.tensor.matmul(o_ps[0:64, :], v_sb[:, k0:k0 + 64],
                                     p_sb[:, 0, :], start=fl, stop=ll)
                    nc.tensor.matmul(o_ps[64:128, :], v_sb[:, k0 + 64:k0 + TK],
                                     p_sb[:, 1, :], start=fl, stop=ll)
                    nc.tensor.matmul(d_ps[0:64, :], ones_sb[:],
                                     p_sb[:, 0, :], start=fl, stop=ll)
                    nc.tensor.matmul(d_ps[64:128, :], ones_sb[:],
                                     p_sb[:, 1, :], start=fl, stop=ll)

                rec = epi.tile([128, TQ], F32, tag="rec")
                nc.vector.reciprocal(rec[:], d_ps[:])
                o_n = epi.tile([128, TQ], F32R, tag="on")
                nc.vector.tensor_mul(o_n[:], o_ps[:], rec[:])

                if 'out' not in phases:
                    continue
                y_sb = ystage.tile([128, 4, C], F32, tag="y")
                for m in range(4):
                    for n in range(2):
                        y_ps = ps_y.tile([128, TQ], F32, tag="y")
                        nc.tensor.matmul(y_ps[:], r(o_n[:, m * 128:(m + 1) * 128]),
                                         r(wo_sb[:, n * TQ:(n + 1) * TQ]))
                        nc.vector.tensor_copy(
                            y_sb[:, m, n * TQ:(n + 1) * TQ], y_ps[:])
                nc.sync.dma_start(y_r[b * NBLK + i], y_sb[:])


# ---------------------------------------------------------------------------
# Host wrapper
# ---------------------------------------------------------------------------

_CACHE = {}


def _prep_inputs(x, Wq, Wk, Wv, Wo):
    xt = np.ascontiguousarray(x.reshape(BT, C).T)            # [C, BT]
    import ml_dtypes
    mask = np.zeros((4, TK, TQ), ml_dtypes.bfloat16)
    for jj in range(4):
        for p in range(TK):
            lo = 128 * jj + p
            if lo < TQ:
                mask[jj, p, lo:] = 1.0
    in_maps = []
    for c in range(N_CORES):
        r0 = c * DPC
        in_maps.append({
            "xt": xt,
            "wq": np.ascontiguousarray(Wq[r0:r0 + DPC, :].T),
            "wk": np.ascontiguousarray(Wk[r0:r0 + DPC, :].T),
            "wv": np.ascontiguousarray(Wv[r0:r0 + DPC, :].T),
            "wo": np.ascontiguousarray(Wo[:, r0:r0 + DPC].T),
            "mask": mask,
            "ones": np.ones((128, 64), ml_dtypes.bfloat16),
        })
    return in_maps


def kernel(x, Wq, Wk, Wv, Wo, bo):
    x = np.asarray(x, np.float32)
    Wq = np.asarray(Wq, np.float32)
    Wk = np.asarray(Wk, np.float32)
    Wv = np.asarray(Wv, np.float32)
    Wo = np.asarray(Wo, np.float32)
    bo = np.asarray(bo, np.float32)

    if "nc" not in _CACHE:
        _CACHE["nc"] = build_kernel()
    nc = _CACHE["nc"]

    in_maps = _prep_inputs(x, Wq, Wk, Wv, Wo)
    res = run_bass_kernel_spmd(nc, in_maps, core_ids=list(range(N_CORES)))
    acc = np.zeros((BT, C), np.float64)
    for r in res.results:
        acc += r["y"]
    out = (acc + bo).astype(np.float32)
    return out.reshape(B, T, C)


# revision 17
# speedup vs baseline: 2.6761x; 1.1957x over previous
"""Multi-head causal self-attention on 8 Trainium2 NeuronCores.

Sharding: tensor-parallel over heads. 16 heads / 8 cores = 2 heads per core.
Each core computes Q/K/V projections for its 2 heads (full batch/seq),
causal attention for those heads, and a partial output projection
y_c = O_c @ Wo[:, cols_c].T. The host sums the 8 partials and adds the bias.

Device layout choices (per core):
  - Host feeds x pre-transposed: xT [1024, 4096]  (c, b*t).
  - Q^T, K^T stored [128(d of 2 heads), t] so the S^T = K @ Q^T matmul pair
    packs both heads onto the PE array via row tiling (K=64 each).
  - Scores kept transposed S^T [tk, tq]; softmax without max subtraction
    (|S| <= ~3 for these inputs, exp is safe), denominators via an
    all-ones stationary matmul, normalization after the PV matmul.
  - Causal masking: fully-masked (tk > all tq) tiles skipped; the 4
    diagonal-crossing [128 tk, 512 tq] tiles per query block are masked
    multiplicatively after exp with precomputed 0/1 masks.
"""

import json
import numpy as np

import concourse.bass as bass
import concourse.tile as tile
from concourse import mybir
from concourse.bass_utils import run_bass_kernel_spmd

B, T, C = 2, 2048, 1024
H, D = 16, 64
N_CORES = 8
HPC = H // N_CORES          # heads per core (2)
DPC = HPC * D               # head-dim per core (128)
BT = B * T                  # 4096
KCH = C // 128              # contraction chunks for projections (8)
TQ = 512                    # query-block width (PSUM bank)
TK = 128                    # key-tile height (partitions)
NBLK = T // TQ              # query blocks per batch (4)
F32 = mybir.dt.float32
BF16 = mybir.dt.bfloat16

# ---------------------------------------------------------------------------
# Walrus in this container rejects instructions carrying more than one sync
# wait ("Too many sync wait commands"). Tile's kernel-tail drain carries
# several. Hoist all but the last wait of any instruction onto fresh NoOps
# inserted immediately before it on the same engine (preserves per-engine
# program order, hence semantics).
# ---------------------------------------------------------------------------

def _split_multi_waits(raw: bytes) -> bytes:
    d = json.loads(raw)

    def fix(insts):
        out = []
        for ins in insts:
            waits = (ins.get('sync_info') or {}).get('on_wait') or []
            if len(waits) > 1:
                for i, w in enumerate(waits[:-1]):
                    out.append({
                        'debug': ins.get('debug'),
                        'engine': ins['engine'],
                        'ins': [], 'outs': [],
                        'name': f"{ins['name']}-w{i}",
                        'opcode': 'NoOp',
                        'sync_info': {'on_update': [], 'on_wait': [w]},
                    })
                ins['sync_info']['on_wait'] = waits[-1:]
            out.append(ins)
        return out

    def walk(obj):
        if isinstance(obj, dict):
            if isinstance(obj.get('instructions'), list):
                obj['instructions'] = fix(obj['instructions'])
            for v in obj.values():
                walk(v)
        elif isinstance(obj, list):
            for v in obj:
                walk(v)

    for f in d.get('functions', []):
        walk(f.get('blocks'))
    return json.dumps(d).encode()


def _install_bir_patch(nc):
    orig = nc.to_json_bytes
    nc.to_json_bytes = lambda: _split_multi_waits(orig())


# ---------------------------------------------------------------------------
# Device kernel (SPMD; per-core inputs differ only in weight slices)
# ---------------------------------------------------------------------------

def build_kernel(nreps=1, phases=('proj', 'attn', 'out')):
    nc = bass.Bass("TRN2", target_bir_lowering=False, debug=False)
    xt = nc.dram_tensor("xt", [C, BT], F32, kind="ExternalInput").ap()
    wq = nc.dram_tensor("wq", [C, DPC], F32, kind="ExternalInput").ap()
    wk = nc.dram_tensor("wk", [C, DPC], F32, kind="ExternalInput").ap()
    wv = nc.dram_tensor("wv", [C, DPC], F32, kind="ExternalInput").ap()
    wo = nc.dram_tensor("wo", [DPC, C], F32, kind="ExternalInput").ap()
    msk = nc.dram_tensor("mask", [4, TK, TQ], BF16, kind="ExternalInput").ap()
    one = nc.dram_tensor("ones", [128, 64], BF16, kind="ExternalInput").ap()
    y = nc.dram_tensor("y", [BT, C], BF16, kind="ExternalOutput").ap()

    xt_r = xt.rearrange("(k p) t -> p k t", p=128)          # [128, 8, 4096]
    wq_r = wq.rearrange("(k p) d -> p k d", p=128)          # [128, 8, 128]
    wk_r = wk.rearrange("(k p) d -> p k d", p=128)
    wv_r = wv.rearrange("(k p) d -> p k d", p=128)
    y_r = y.rearrange("(blk m p) c -> blk p m c", m=4, p=128)  # [8, 128, 4, 1024]

    with tile.TileContext(nc) as tc:
        for _ in range(nreps):
            _build_body(nc, tc, xt_r, wq_r, wk_r, wv_r, wo, msk, one, y_r, phases)
    _install_bir_patch(nc)
    return nc


def _build_body(nc, tc, xt_r, wq_r, wk_r, wv_r, wo, msk, one, y_r, phases=('proj', 'attn', 'out')):
    from contextlib import ExitStack
    from concourse.masks import make_identity

    F32R = mybir.dt.float32r

    def r(ap):
        return ap.bitcast(F32R)

    ctx = ExitStack()
    with ctx:
        const = ctx.enter_context(tc.tile_pool(name="const", bufs=1))
        xt_pool = ctx.enter_context(tc.tile_pool(name="xt", bufs=3))
        qkv = ctx.enter_context(tc.tile_pool(name="qkv", bufs=1))
        vt_pool = ctx.enter_context(tc.tile_pool(name="vt", bufs=2))
        p_pool = ctx.enter_context(tc.tile_pool(name="p", bufs=3))
        epi = ctx.enter_context(tc.tile_pool(name="epi", bufs=2))
        ystage = ctx.enter_context(tc.tile_pool(name="ystage", bufs=2))
        # 8 PSUM banks total: s-pool 2 slots x 2 banks (also serves the
        # 1-bank proj/out-proj tiles), o and d 1 slot x 2 banks each.
        ps_s = ctx.enter_context(tc.tile_pool(name="ps_s", bufs=2, space="PSUM"))
        ps_o = ctx.enter_context(tc.tile_pool(name="ps_o", bufs=1, space="PSUM"))
        ps_d = ctx.enter_context(tc.tile_pool(name="ps_d", bufs=1, space="PSUM"))
        ps_y = ctx.enter_context(tc.tile_pool(name="ps_y", bufs=2, space="PSUM"))

        # --- constants ---
        wq_sb = const.tile([128, KCH, DPC], F32R, tag="wq")
        wk_sb = const.tile([128, KCH, DPC], F32R, tag="wk")
        wv_sb = const.tile([128, KCH, DPC], F32R, tag="wv")
        wo_sb = const.tile([128, C], F32R, tag="wo")
        mask_sb = const.tile([128, 4, TQ], BF16, tag="mask")
        ones_sb = const.tile([128, 64], BF16, tag="ones")
        ident_sb = const.tile([128, 128], F32, tag="ident")
        nc.sync.dma_start(wq_sb[:], r(wq_r[:]))
        nc.sync.dma_start(wk_sb[:], r(wk_r[:]))
        nc.sync.dma_start(wv_sb[:], r(wv_r[:]))
        nc.sync.dma_start(wo_sb[:], r(wo[:]))
        nc.sync.dma_start(mask_sb[:], msk.rearrange("j p q -> p j q"))
        nc.sync.dma_start(ones_sb[:], one[:])
        make_identity(nc, ident_sb[:])

        # --- persistent Q^T / K^T / V tiles, separate per batch so batch
        # b+1's projections overlap batch b's attention ---
        qt_b = [qkv.tile([128, T], F32R, name=f"qt{b}", tag=f"qt{b}") for b in range(B)]
        kt_b = [qkv.tile([128, T], F32R, name=f"kt{b}", tag=f"kt{b}") for b in range(B)]
        v_b = [qkv.tile([128, T], BF16, name=f"v{b}", tag=f"v{b}") for b in range(B)]

        for b in range(B):
            qt_sb, kt_sb, v_sb = qt_b[b], kt_b[b], v_b[b]
            # --- projections for batch b ---
            for tchunk in range(T // TQ) if 'proj' in phases else []:
                t0 = b * T + tchunk * TQ
                xt_sb = xt_pool.tile([128, KCH, TQ], F32R, tag="xt")
                nc.sync.dma_start(xt_sb[:], r(xt_r[:, :, t0:t0 + TQ]))

                for w_sb, dst in ((wq_sb, qt_sb), (wk_sb, kt_sb)):
                    ps = ps_s.tile([128, TQ], F32, tag="s")
                    for k in range(KCH):
                        nc.tensor.matmul(ps[:], r(w_sb[:, k, :]), xt_sb[:, k, :],
                                         start=(k == 0), stop=(k == KCH - 1))
                    nc.vector.tensor_copy(
                        dst[:, tchunk * TQ:(tchunk + 1) * TQ], ps[:])

                # V^T in PSUM, copy to SBUF, then PE-transpose to [t, d]
                ps = ps_s.tile([128, TQ], F32, tag="s")
                for k in range(KCH):
                    nc.tensor.matmul(ps[:], wv_sb[:, k, :], xt_sb[:, k, :],
                                     start=(k == 0), stop=(k == KCH - 1))
                vt_sb = vt_pool.tile([128, TQ], F32, tag="vt")
                nc.vector.tensor_copy(vt_sb[:], ps[:])
                ps = ps_s.tile([128, TQ], F32, tag="s")
                for m in range(4):
                    nc.tensor.transpose(ps[:, m * 128:(m + 1) * 128],
                                        vt_sb[:, m * 128:(m + 1) * 128],
                                        ident_sb[:])
                nc.vector.tensor_copy(
                    v_sb[:, tchunk * TQ:(tchunk + 1) * TQ], ps[:])

            # --- attention + partial out-projection for batch b ---
            for i in range(NBLK) if 'attn' in phases else []:
                q0 = i * TQ
                njt = 4 * i + 4           # needed key tiles (causal)
                o_ps = ps_o.tile([128, TQ], F32, tag="o")
                d_ps = ps_d.tile([128, TQ], F32, tag="d")
                for j in range(njt):
                    # S^T pair: head A on PE rows 0-63 -> psum half 0, head B
                    # on rows 64-127 -> half 1 (row-tiled, runs concurrently).
                    # One exp covers both heads (1024-wide batch).
                    k0 = j * TK
                    s_ps = ps_s.tile([128, 2, TQ], F32, tag="s")
                    p_sb = p_pool.tile([128, 2, TQ], BF16, tag="p")
                    nc.tensor.matmul(s_ps[:, 0, :],
                                     kt_sb[0:64, k0:k0 + TK],
                                     qt_sb[0:64, q0:q0 + TQ])
                    nc.tensor.matmul(s_ps[:, 1, :],
                                     kt_sb[64:128, k0:k0 + TK],
                                     qt_sb[64:128, q0:q0 + TQ])
                    nc.scalar.activation(p_sb[:], s_ps[:],
                                         mybir.ActivationFunctionType.Exp,
                                         scale=0.125)
                    if j >= 4 * i:            # diagonal-crossing tile
                        jj = j - 4 * i
                        nc.vector.tensor_mul(p_sb[:, 0, :], p_sb[:, 0, :],
                                             mask_sb[:, jj, :])
                        nc.vector.tensor_mul(p_sb[:, 1, :], p_sb[:, 1, :],
                                             mask_sb[:, jj, :])
                    fl = (j == 0)
                    ll = (j == njt - 1)
                    # bf16 PV + denominator matmuls, column-tiled so the two
                    # heads run concurrently on separate PE column halves.
                    nc.tensor.matmul(o_ps[0:64, :], v_sb[:, k0:k0 + 64],
                                     p_sb[:, 0, :], start=fl, stop=ll)
                    nc.tensor.matmul(o_ps[64:128, :], v_sb[:, k0 + 64:k0 + TK],
                                     p_sb[:, 1, :], start=fl, stop=ll)
                    nc.tensor.matmul(d_ps[0:64, :], ones_sb[:],
                                     p_sb[:, 0, :], start=fl, stop=ll)
                    nc.tensor.matmul(d_ps[64:128, :], ones_sb[:],
                                     p_sb[:, 1, :], start=fl, stop=ll)

                rec = epi.tile([128, TQ], F32, tag="rec")
                nc.vector.reciprocal(rec[:], d_ps[:])
                o_n = epi.tile([128, TQ], F32R, tag="on")
                nc.vector.tensor_mul(o_n[:], o_ps[:], rec[:])

                if 'out' not in phases:
                    continue
                y_sb = ystage.tile([128, 4, C], BF16, tag="y")
                for m in range(4):
                    for n in range(2):
                        y_ps = ps_y.tile([128, TQ], F32, tag="y")
                        nc.tensor.matmul(y_ps[:], r(o_n[:, m * 128:(m + 1) * 128]),
                                         r(wo_sb[:, n * TQ:(n + 1) * TQ]))
                        nc.vector.tensor_copy(
                            y_sb[:, m, n * TQ:(n + 1) * TQ], y_ps[:])
                nc.sync.dma_start(y_r[b * NBLK + i], y_sb[:])


# ---------------------------------------------------------------------------
# Host wrapper
# ---------------------------------------------------------------------------

_CACHE = {}


def _prep_inputs(x, Wq, Wk, Wv, Wo):
    xt = np.ascontiguousarray(x.reshape(BT, C).T)            # [C, BT]
    import ml_dtypes
    mask = np.zeros((4, TK, TQ), ml_dtypes.bfloat16)
    for jj in range(4):
        for p in range(TK):
            lo = 128 * jj + p
            if lo < TQ:
                mask[jj, p, lo:] = 1.0
    in_maps = []
    for c in range(N_CORES):
        r0 = c * DPC
        in_maps.append({
            "xt": xt,
            "wq": np.ascontiguousarray(Wq[r0:r0 + DPC, :].T),
            "wk": np.ascontiguousarray(Wk[r0:r0 + DPC, :].T),
            "wv": np.ascontiguousarray(Wv[r0:r0 + DPC, :].T),
            "wo": np.ascontiguousarray(Wo[:, r0:r0 + DPC].T),
            "mask": mask,
            "ones": np.ones((128, 64), ml_dtypes.bfloat16),
        })
    return in_maps


def kernel(x, Wq, Wk, Wv, Wo, bo):
    x = np.asarray(x, np.float32)
    Wq = np.asarray(Wq, np.float32)
    Wk = np.asarray(Wk, np.float32)
    Wv = np.asarray(Wv, np.float32)
    Wo = np.asarray(Wo, np.float32)
    bo = np.asarray(bo, np.float32)

    if "nc" not in _CACHE:
        _CACHE["nc"] = build_kernel()
    nc = _CACHE["nc"]

    in_maps = _prep_inputs(x, Wq, Wk, Wv, Wo)
    res = run_bass_kernel_spmd(nc, in_maps, core_ids=list(range(N_CORES)))
    acc = np.zeros((BT, C), np.float64)
    for r in res.results:
        acc += r["y"]
    out = (acc + bo).astype(np.float32)
    return out.reshape(B, T, C)


# revision 19
# speedup vs baseline: 2.7991x; 1.0460x over previous
"""Multi-head causal self-attention on 8 Trainium2 NeuronCores.

Sharding: tensor-parallel over heads. 16 heads / 8 cores = 2 heads per core.
Each core computes Q/K/V projections for its 2 heads (full batch/seq),
causal attention for those heads, and a partial output projection
y_c = O_c @ Wo[:, cols_c].T. The host sums the 8 partials and adds the bias.

Device layout choices (per core):
  - Host feeds x pre-transposed: xT [1024, 4096]  (c, b*t).
  - Q^T, K^T stored [128(d of 2 heads), t] so the S^T = K @ Q^T matmul pair
    packs both heads onto the PE array via row tiling (K=64 each).
  - Scores kept transposed S^T [tk, tq]; softmax without max subtraction
    (|S| <= ~3 for these inputs, exp is safe), denominators via an
    all-ones stationary matmul, normalization after the PV matmul.
  - Causal masking: fully-masked (tk > all tq) tiles skipped; the 4
    diagonal-crossing [128 tk, 512 tq] tiles per query block are masked
    multiplicatively after exp with precomputed 0/1 masks.
"""

import json
import numpy as np

import concourse.bass as bass
import concourse.tile as tile
from concourse import mybir
from concourse.bass_utils import run_bass_kernel_spmd

B, T, C = 2, 2048, 1024
H, D = 16, 64
N_CORES = 8
HPC = H // N_CORES          # heads per core (2)
DPC = HPC * D               # head-dim per core (128)
BT = B * T                  # 4096
KCH = C // 128              # contraction chunks for projections (8)
TQ = 512                    # query-block width (PSUM bank)
TK = 128                    # key-tile height (partitions)
NBLK = T // TQ              # query blocks per batch (4)
F32 = mybir.dt.float32
BF16 = mybir.dt.bfloat16

# ---------------------------------------------------------------------------
# Walrus in this container rejects instructions carrying more than one sync
# wait ("Too many sync wait commands"). Tile's kernel-tail drain carries
# several. Hoist all but the last wait of any instruction onto fresh NoOps
# inserted immediately before it on the same engine (preserves per-engine
# program order, hence semantics).
# ---------------------------------------------------------------------------

def _split_multi_waits(raw: bytes) -> bytes:
    d = json.loads(raw)

    def fix(insts):
        out = []
        for ins in insts:
            waits = (ins.get('sync_info') or {}).get('on_wait') or []
            if len(waits) > 1:
                for i, w in enumerate(waits[:-1]):
                    out.append({
                        'debug': ins.get('debug'),
                        'engine': ins['engine'],
                        'ins': [], 'outs': [],
                        'name': f"{ins['name']}-w{i}",
                        'opcode': 'NoOp',
                        'sync_info': {'on_update': [], 'on_wait': [w]},
                    })
                ins['sync_info']['on_wait'] = waits[-1:]
            out.append(ins)
        return out

    def walk(obj):
        if isinstance(obj, dict):
            if isinstance(obj.get('instructions'), list):
                obj['instructions'] = fix(obj['instructions'])
            for v in obj.values():
                walk(v)
        elif isinstance(obj, list):
            for v in obj:
                walk(v)

    for f in d.get('functions', []):
        walk(f.get('blocks'))
    return json.dumps(d).encode()


def _install_bir_patch(nc):
    orig = nc.to_json_bytes
    nc.to_json_bytes = lambda: _split_multi_waits(orig())


# ---------------------------------------------------------------------------
# Device kernel (SPMD; per-core inputs differ only in weight slices)
# ---------------------------------------------------------------------------

def build_kernel(nreps=1, phases=('proj', 'attn', 'out')):
    nc = bass.Bass("TRN2", target_bir_lowering=False, debug=False)
    xt = nc.dram_tensor("xt", [C, BT], F32, kind="ExternalInput").ap()
    wq = nc.dram_tensor("wq", [C, DPC], F32, kind="ExternalInput").ap()
    wk = nc.dram_tensor("wk", [C, DPC], F32, kind="ExternalInput").ap()
    wv = nc.dram_tensor("wv", [C, DPC], F32, kind="ExternalInput").ap()
    wo = nc.dram_tensor("wo", [DPC, C], F32, kind="ExternalInput").ap()
    msk = nc.dram_tensor("mask", [4, TK, TQ], BF16, kind="ExternalInput").ap()
    one = nc.dram_tensor("ones", [128, 64], BF16, kind="ExternalInput").ap()
    y = nc.dram_tensor("y", [BT, C], BF16, kind="ExternalOutput").ap()

    xt_r = xt.rearrange("(k p) t -> p k t", p=128)          # [128, 8, 4096]
    wq_r = wq.rearrange("(k p) d -> p k d", p=128)          # [128, 8, 128]
    wk_r = wk.rearrange("(k p) d -> p k d", p=128)
    wv_r = wv.rearrange("(k p) d -> p k d", p=128)
    y_r = y.rearrange("(blk m p) c -> blk p m c", m=4, p=128)  # [8, 128, 4, 1024]

    with tile.TileContext(nc) as tc:
        for _ in range(nreps):
            _build_body(nc, tc, xt_r, wq_r, wk_r, wv_r, wo, msk, one, y_r, phases)
    _install_bir_patch(nc)
    return nc


def _build_body(nc, tc, xt_r, wq_r, wk_r, wv_r, wo, msk, one, y_r, phases=('proj', 'attn', 'out')):
    from contextlib import ExitStack
    from concourse.masks import make_identity

    F32R = mybir.dt.float32r

    def r(ap):
        return ap.bitcast(F32R)

    ctx = ExitStack()
    with ctx:
        const = ctx.enter_context(tc.tile_pool(name="const", bufs=1))
        xt_pool = ctx.enter_context(tc.tile_pool(name="xt", bufs=3))
        qkv = ctx.enter_context(tc.tile_pool(name="qkv", bufs=1))
        vt_pool = ctx.enter_context(tc.tile_pool(name="vt", bufs=2))
        p_pool = ctx.enter_context(tc.tile_pool(name="p", bufs=3))
        epi = ctx.enter_context(tc.tile_pool(name="epi", bufs=2))
        ystage = ctx.enter_context(tc.tile_pool(name="ystage", bufs=2))
        # 8 PSUM banks total: s-pool 2 slots x 2 banks (also serves the
        # 1-bank proj/out-proj tiles), o and d 1 slot x 2 banks each.
        ps_s = ctx.enter_context(tc.tile_pool(name="ps_s", bufs=2, space="PSUM"))
        ps_o = ctx.enter_context(tc.tile_pool(name="ps_o", bufs=1, space="PSUM"))
        ps_d = ctx.enter_context(tc.tile_pool(name="ps_d", bufs=1, space="PSUM"))
        ps_y = ctx.enter_context(tc.tile_pool(name="ps_y", bufs=2, space="PSUM"))

        # --- constants ---
        wq_sb = const.tile([128, KCH, DPC], F32R, tag="wq")
        wk_sb = const.tile([128, KCH, DPC], F32R, tag="wk")
        wv_sb = const.tile([128, KCH, DPC], F32R, tag="wv")
        wo_sb = const.tile([128, C], F32R, tag="wo")
        mask_sb = const.tile([128, 4, TQ], BF16, tag="mask")
        ones_sb = const.tile([128, 64], BF16, tag="ones")
        ident_sb = const.tile([128, 128], F32, tag="ident")
        nc.sync.dma_start(wq_sb[:], r(wq_r[:]))
        nc.sync.dma_start(wk_sb[:], r(wk_r[:]))
        nc.sync.dma_start(wv_sb[:], r(wv_r[:]))
        nc.sync.dma_start(wo_sb[:], r(wo[:]))
        nc.sync.dma_start(mask_sb[:], msk.rearrange("j p q -> p j q"))
        nc.sync.dma_start(ones_sb[:], one[:])
        make_identity(nc, ident_sb[:])

        # --- persistent Q^T / K^T / V tiles, separate per batch so batch
        # b+1's projections overlap batch b's attention ---
        qt_b = [qkv.tile([128, T], F32R, name=f"qt{b}", tag=f"qt{b}") for b in range(B)]
        kt_b = [qkv.tile([128, T], F32R, name=f"kt{b}", tag=f"kt{b}") for b in range(B)]
        v_b = [qkv.tile([128, T], BF16, name=f"v{b}", tag=f"v{b}") for b in range(B)]

        for b in range(B):
            qt_sb, kt_sb, v_sb = qt_b[b], kt_b[b], v_b[b]
            # --- projections for batch b ---
            for tchunk in range(T // TQ) if 'proj' in phases else []:
                t0 = b * T + tchunk * TQ
                xt_sb = xt_pool.tile([128, KCH, TQ], F32R, tag="xt")
                nc.sync.dma_start(xt_sb[:], r(xt_r[:, :, t0:t0 + TQ]))

                for w_sb, dst in ((wq_sb, qt_sb), (wk_sb, kt_sb)):
                    ps = ps_s.tile([128, TQ], F32, tag="s")
                    for k in range(KCH):
                        nc.tensor.matmul(ps[:], r(w_sb[:, k, :]), xt_sb[:, k, :],
                                         start=(k == 0), stop=(k == KCH - 1))
                    nc.vector.tensor_copy(
                        dst[:, tchunk * TQ:(tchunk + 1) * TQ], ps[:])

                # V^T in PSUM, copy to SBUF, then PE-transpose to [t, d]
                ps = ps_s.tile([128, TQ], F32, tag="s")
                for k in range(KCH):
                    nc.tensor.matmul(ps[:], wv_sb[:, k, :], xt_sb[:, k, :],
                                     start=(k == 0), stop=(k == KCH - 1))
                vt_sb = vt_pool.tile([128, TQ], F32, tag="vt")
                nc.vector.tensor_copy(vt_sb[:], ps[:])
                ps = ps_s.tile([128, TQ], F32, tag="s")
                for m in range(4):
                    nc.tensor.transpose(ps[:, m * 128:(m + 1) * 128],
                                        vt_sb[:, m * 128:(m + 1) * 128],
                                        ident_sb[:])
                nc.vector.tensor_copy(
                    v_sb[:, tchunk * TQ:(tchunk + 1) * TQ], ps[:])

            # --- attention + partial out-projection for batch b ---
            for i in range(NBLK) if 'attn' in phases else []:
                q0 = i * TQ
                njt = 4 * i + 4           # needed key tiles (causal)
                o_ps = ps_o.tile([128, TQ], F32, tag="o")
                d_ps = ps_d.tile([128, TQ], F32, tag="d")
                for j in range(njt):
                    # S^T pair: head A on PE rows 0-63 -> psum half 0, head B
                    # on rows 64-127 -> half 1 (row-tiled, runs concurrently).
                    # One exp covers both heads (1024-wide batch).
                    k0 = j * TK
                    s_ps = ps_s.tile([128, 2, TQ], F32, tag="s")
                    p_sb = p_pool.tile([128, 2, TQ], BF16, tag="p")
                    nc.tensor.matmul(s_ps[:, 0, :],
                                     kt_sb[0:64, k0:k0 + TK],
                                     qt_sb[0:64, q0:q0 + TQ])
                    nc.tensor.matmul(s_ps[:, 1, :],
                                     kt_sb[64:128, k0:k0 + TK],
                                     qt_sb[64:128, q0:q0 + TQ])
                    nc.scalar.activation(p_sb[:], s_ps[:],
                                         mybir.ActivationFunctionType.Exp,
                                         scale=0.125)
                    if j >= 4 * i:            # diagonal-crossing tile
                        jj = j - 4 * i
                        nc.gpsimd.tensor_mul(p_sb[:, 0, :], p_sb[:, 0, :],
                                             mask_sb[:, jj, :])
                        nc.gpsimd.tensor_mul(p_sb[:, 1, :], p_sb[:, 1, :],
                                             mask_sb[:, jj, :])
                    fl = (j == 0)
                    ll = (j == njt - 1)
                    # bf16 PV + denominator matmuls, column-tiled so the two
                    # heads run concurrently on separate PE column halves.
                    nc.tensor.matmul(o_ps[0:64, :], v_sb[:, k0:k0 + 64],
                                     p_sb[:, 0, :], start=fl, stop=ll)
                    nc.tensor.matmul(o_ps[64:128, :], v_sb[:, k0 + 64:k0 + TK],
                                     p_sb[:, 1, :], start=fl, stop=ll)
                    nc.tensor.matmul(d_ps[0:64, :], ones_sb[:],
                                     p_sb[:, 0, :], start=fl, stop=ll)
                    nc.tensor.matmul(d_ps[64:128, :], ones_sb[:],
                                     p_sb[:, 1, :], start=fl, stop=ll)

                lnd = epi.tile([128, TQ], F32, tag="lnd")
                nc.scalar.activation(lnd[:], d_ps[:],
                                     mybir.ActivationFunctionType.Ln)
                rec = epi.tile([128, TQ], F32, tag="rec")
                nc.scalar.activation(rec[:], lnd[:],
                                     mybir.ActivationFunctionType.Exp,
                                     scale=-1.0)
                o_n = epi.tile([128, TQ], F32R, tag="on")
                nc.vector.tensor_mul(o_n[:], o_ps[:], rec[:])

                if 'out' not in phases:
                    continue
                y_sb = ystage.tile([128, 4, C], BF16, tag="y")
                for m in range(4):
                    for n in range(2):
                        y_ps = ps_y.tile([128, TQ], F32, tag="y")
                        nc.tensor.matmul(y_ps[:], r(o_n[:, m * 128:(m + 1) * 128]),
                                         r(wo_sb[:, n * TQ:(n + 1) * TQ]))
                        if (m + n) % 2 == 0:
                            nc.vector.tensor_copy(
                                y_sb[:, m, n * TQ:(n + 1) * TQ], y_ps[:])
                        else:
                            nc.scalar.copy(
                                y_sb[:, m, n * TQ:(n + 1) * TQ], y_ps[:])
                nc.sync.dma_start(y_r[b * NBLK + i], y_sb[:])


# ---------------------------------------------------------------------------
# Host wrapper
# ---------------------------------------------------------------------------

_CACHE = {}


def _prep_inputs(x, Wq, Wk, Wv, Wo):
    xt = np.ascontiguousarray(x.reshape(BT, C).T)            # [C, BT]
    import ml_dtypes
    mask = np.zeros((4, TK, TQ), ml_dtypes.bfloat16)
    for jj in range(4):
        for p in range(TK):
            lo = 128 * jj + p
            if lo < TQ:
                mask[jj, p, lo:] = 1.0
    in_maps = []
    for c in range(N_CORES):
        r0 = c * DPC
        in_maps.append({
            "xt": xt,
            "wq": np.ascontiguousarray(Wq[r0:r0 + DPC, :].T),
            "wk": np.ascontiguousarray(Wk[r0:r0 + DPC, :].T),
            "wv": np.ascontiguousarray(Wv[r0:r0 + DPC, :].T),
            "wo": np.ascontiguousarray(Wo[:, r0:r0 + DPC].T),
            "mask": mask,
            "ones": np.ones((128, 64), ml_dtypes.bfloat16),
        })
    return in_maps


def kernel(x, Wq, Wk, Wv, Wo, bo):
    x = np.asarray(x, np.float32)
    Wq = np.asarray(Wq, np.float32)
    Wk = np.asarray(Wk, np.float32)
    Wv = np.asarray(Wv, np.float32)
    Wo = np.asarray(Wo, np.float32)
    bo = np.asarray(bo, np.float32)

    if "nc" not in _CACHE:
        _CACHE["nc"] = build_kernel()
    nc = _CACHE["nc"]

    in_maps = _prep_inputs(x, Wq, Wk, Wv, Wo)
    res = run_bass_kernel_spmd(nc, in_maps, core_ids=list(range(N_CORES)))
    acc = np.zeros((BT, C), np.float64)
    for r in res.results:
        acc += r["y"]
    out = (acc + bo).astype(np.float32)
    return out.reshape(B, T, C)
